# revision 1
# baseline (speedup 1.0000x reference)
"""Additive (Bahdanau) attention kernel for 8 TRN2 NeuronCores.

reference:
    q = query @ wq.T + bq            # [B, Lq, H]
    k = key  @ wk.T + bk             # [B, Lk, H]
    scores[b,qi,ki] = sum_h wv[h] * tanh(q[b,qi,h] + k[b,ki,h]) + bv
    out = softmax(scores, -1) @ value

Sharding: data-parallel over (B=4) x (Lq halves) -> 8 cores, each core
computes out[b, qh*256:(qh+1)*256, :] fully locally (no collectives).

Per-core device algorithm (shapes per core: Lq'=256, Lk=512, H=256):
  - projections via PE (host supplies pre-transposed queryT/keyT/wqT/wkT)
    -> qT [h, qi], kT [h, ki] with h on partitions (2 chunks of 128)
  - for each qi: feats_c = tanh(kT_c + qT_c[:, qi]) fused on the Scalar
    engine (per-partition bias); weighted reduce over h on the PE using a
    strided one-hot wv stationary so row qi of the scores PSUM tile is
    sum_h wv[h]*feats[h, :] (256-matmul accumulation per 128-qi tile).
  - softmax along free axis, attn transposed on PE, attn @ value on PE,
    1/rowsum folded into the output scale.

bv is omitted: it cancels in the softmax.
"""

import os
import sys

import numpy as np

for _p in ("/root/.axon_site", "/root/.axon_site/_ro/trn_rl_repo", "/opt/trn_rl_repo"):
    if os.path.isdir(_p) and _p not in sys.path:
        sys.path.append(_p)

import concourse.bacc as bacc
import concourse.bass as bass
import concourse.mybir as mybir
import concourse.tile as tile
from concourse.bass_utils import run_bass_kernel_spmd

B, LQ, LK = 4, 512, 512
QS, KS, H, DV = 512, 512, 256, 512
NCORES = 8
LQS = B * LQ // NCORES  # 256 query rows per core
QT = 128  # qi tile (partition dim)
F32 = mybir.dt.float32
AF = mybir.ActivationFunctionType


def build():
    nc = bacc.Bacc("TRN2", target_bir_lowering=False, debug=False)

    queryT = nc.dram_tensor("queryT", [QS, LQS], F32, kind="ExternalInput")
    keyT = nc.dram_tensor("keyT", [KS, LK], F32, kind="ExternalInput")
    value = nc.dram_tensor("value", [LK, DV], F32, kind="ExternalInput")
    wqT = nc.dram_tensor("wqT", [QS, H], F32, kind="ExternalInput")
    wkT = nc.dram_tensor("wkT", [KS, H], F32, kind="ExternalInput")
    bqc = nc.dram_tensor("bqc", [128, 2], F32, kind="ExternalInput")
    bkc = nc.dram_tensor("bkc", [128, 2], F32, kind="ExternalInput")
    ohwv = nc.dram_tensor("ohwv", [128, 514], F32, kind="ExternalInput")
    ident = nc.dram_tensor("ident", [128, 128], F32, kind="ExternalInput")
    out = nc.dram_tensor("out", [LQS, DV], F32, kind="ExternalOutput")

    with tile.TileContext(nc) as tc:
        with (
            tc.tile_pool(name="const", bufs=1) as constp,
            tc.tile_pool(name="feat", bufs=6) as featp,
            tc.tile_pool(name="sm", bufs=2) as smp,
            tc.tile_pool(name="ps_s", bufs=2, space="PSUM") as ps_s,
            tc.tile_pool(name="ps_t", bufs=2, space="PSUM") as ps_t,
            tc.tile_pool(name="ps_o", bufs=2, space="PSUM") as ps_o,
            tc.tile_pool(name="ps_p", bufs=2, space="PSUM") as ps_p,
        ):
            # ---- loads (d-major chunks of 128 partitions) ----
            qT_d = constp.tile([128, QS // 128, LQS], F32)
            nc.sync.dma_start(qT_d[:], queryT.ap().rearrange("(c p) q -> p c q", p=128))
            kT_d = constp.tile([128, KS // 128, LK], F32)
            nc.sync.dma_start(kT_d[:], keyT.ap().rearrange("(c p) k -> p c k", p=128))
            val = constp.tile([128, LK // 128, DV], F32)
            nc.sync.dma_start(val[:], value.ap().rearrange("(c p) d -> p c d", p=128))
            wq_s = constp.tile([128, QS // 128, H], F32)
            nc.sync.dma_start(wq_s[:], wqT.ap().rearrange("(c p) h -> p c h", p=128))
            wk_s = constp.tile([128, KS // 128, H], F32)
            nc.sync.dma_start(wk_s[:], wkT.ap().rearrange("(c p) h -> p c h", p=128))
            bq_s = constp.tile([128, 2], F32)
            nc.sync.dma_start(bq_s[:], bqc[:, :])
            bk_s = constp.tile([128, 2], F32)
            nc.sync.dma_start(bk_s[:], bkc[:, :])
            oh_s = constp.tile([128, 514], F32)
            nc.sync.dma_start(oh_s[:], ohwv[:, :])
            id_s = constp.tile([128, 128], F32)
            nc.sync.dma_start(id_s[:], ident[:, :])

            # ---- projections: qTt [128, 2, LQS], kTt [128, 2, LK] ----
            qTt = constp.tile([128, 2, LQS], F32)
            kTt = constp.tile([128, 2, LK], F32)
            for hc in range(2):
                pq = ps_p.tile([128, LQS], F32, tag="proj")
                for dc in range(QS // 128):
                    nc.tensor.matmul(
                        pq[:],
                        wq_s[:, dc, hc * 128 : (hc + 1) * 128],
                        qT_d[:, dc, :],
                        start=(dc == 0),
                        stop=(dc == QS // 128 - 1),
                    )
                nc.scalar.activation(
                    qTt[:, hc, :], pq[:], AF.Identity, bias=bq_s[:, hc : hc + 1]
                )
                pk = ps_p.tile([128, LK], F32, tag="proj")
                for dc in range(KS // 128):
                    nc.tensor.matmul(
                        pk[:],
                        wk_s[:, dc, hc * 128 : (hc + 1) * 128],
                        kT_d[:, dc, :],
                        start=(dc == 0),
                        stop=(dc == KS // 128 - 1),
                    )
                nc.scalar.activation(
                    kTt[:, hc, :], pk[:], AF.Identity, bias=bk_s[:, hc : hc + 1]
                )

            # ---- main: scores, softmax, AV per 128-qi tile ----
            for t in range(LQS // QT):
                ps_scores = ps_s.tile([128, LK], F32, tag="scores")
                for qi in range(QT):
                    qg = t * QT + qi
                    for c in range(2):
                        ft = featp.tile([128, LK], F32, tag="feats")
                        nc.scalar.activation(
                            ft[:], kTt[:, c, :], AF.Tanh, bias=qTt[:, c, qg : qg + 1]
                        )
                        nc.tensor.matmul(
                            ps_scores[:],
                            oh_s[:, c * 257 + 128 - qi : c * 257 + 256 - qi],
                            ft[:],
                            start=(qi == 0 and c == 0),
                            stop=(qi == QT - 1 and c == 1),
                        )
                negmax = smp.tile([128, 1], F32, tag="negmax")
                nc.vector.tensor_reduce(
                    negmax[:],
                    ps_scores[:],
                    axis=mybir.AxisListType.X,
                    op=mybir.AluOpType.max,
                    negate=True,
                )
                p = smp.tile([128, LK], F32, tag="p")
                nc.scalar.activation(p[:], ps_scores[:], AF.Exp, bias=negmax[:])
                ssum = smp.tile([128, 1], F32, tag="ssum")
                nc.vector.reduce_sum(ssum[:], p[:], axis=mybir.AxisListType.X)
                rinv = smp.tile([128, 1], F32, tag="rinv")
                nc.vector.reciprocal(rinv[:], ssum[:])

                ps_out = ps_o.tile([128, DV], F32, tag="av")
                for kc in range(LK // 128):
                    ptp = ps_t.tile([128, 128], F32, tag="ptp")
                    nc.tensor.transpose(ptp[:], p[:, kc * 128 : (kc + 1) * 128], id_s[:])
                    pts = featp.tile([128, 128], F32, tag="pts")
                    nc.vector.tensor_copy(pts[:], ptp[:])
                    nc.tensor.matmul(
                        ps_out[:],
                        pts[:],
                        val[:, kc, :],
                        start=(kc == 0),
                        stop=(kc == LK // 128 - 1),
                    )
                outs = smp.tile([128, DV], F32, tag="outs")
                nc.vector.tensor_scalar_mul(outs[:], ps_out[:], rinv[:])
                nc.sync.dma_start(out[t * QT : (t + 1) * QT, :], outs[:])

    nc.compile()
    return nc


_NC_CACHE = None


def _get_nc():
    global _NC_CACHE
    if _NC_CACHE is None:
        _NC_CACHE = build()
    return _NC_CACHE


def _make_in_maps(query, key, value, wq, bq, wk, bk, wv, bv):
    del bv  # cancels in softmax
    f = np.float32
    wqT = np.ascontiguousarray(np.asarray(wq, f).T)  # [QS, H]
    wkT = np.ascontiguousarray(np.asarray(wk, f).T)
    bq = np.asarray(bq, f)
    bk = np.asarray(bk, f)
    wv = np.asarray(wv, f)
    bqc = np.ascontiguousarray(bq.reshape(2, 128).T)  # [128, 2]
    bkc = np.ascontiguousarray(bk.reshape(2, 128).T)
    ohwv = np.zeros((128, 514), f)
    ohwv[:, 128] = wv[:128]
    ohwv[:, 385] = wv[128:]
    ident = np.eye(128, dtype=f)
    in_maps = []
    for core in range(NCORES):
        b, qh = divmod(core, NCORES // B)
        qsl = np.asarray(query[b, qh * LQS : (qh + 1) * LQS], f)  # [LQS, QS]
        in_maps.append(
            {
                "queryT": np.ascontiguousarray(qsl.T),
                "keyT": np.ascontiguousarray(np.asarray(key[b], f).T),
                "value": np.ascontiguousarray(np.asarray(value[b], f)),
                "wqT": wqT,
                "wkT": wkT,
                "bqc": bqc,
                "bkc": bkc,
                "ohwv": ohwv,
                "ident": ident,
            }
        )
    return in_maps


def _assemble(results):
    full = np.empty((B, LQ, DV), np.float32)
    for core in range(NCORES):
        b, qh = divmod(core, NCORES // B)
        full[b, qh * LQS : (qh + 1) * LQS, :] = results[core]["out"]
    return full


def run(inputs, trace=False, tmpdir=None):
    nc = _get_nc()
    in_maps = _make_in_maps(**inputs)
    kw = {}
    if trace:
        kw = dict(trace=True, tmpdir=tmpdir, trace_cores=list(range(NCORES)))
    res = run_bass_kernel_spmd(nc, in_maps, core_ids=list(range(NCORES)), **kw)
    return _assemble(res.results), res


def kernel(**inputs):
    out, _ = run(inputs, trace=False)
    return out


# revision 3
# speedup vs baseline: 1.3385x; 1.3385x over previous
"""Additive (Bahdanau) attention kernel for 8 TRN2 NeuronCores.

reference:
    q = query @ wq.T + bq            # [B, Lq, H]
    k = key  @ wk.T + bk             # [B, Lk, H]
    scores[b,qi,ki] = sum_h wv[h] * tanh(q[b,qi,h] + k[b,ki,h]) + bv
    out = softmax(scores, -1) @ value

Sharding: data-parallel over (B=4) x (Lq halves) -> 8 cores, each core
computes out[b, qh*256:(qh+1)*256, :] fully locally (no collectives).

Per-core device algorithm (shapes per core: Lq'=256, Lk=512, H=256):
  - projections via PE (host supplies pre-transposed queryT/keyT/wqT/wkT)
    -> qT [h, qi], kT [h, ki] with h on partitions (2 chunks of 128)
  - for each qi: feats_c = tanh(kT_c + qT_c[:, qi]) fused on the Scalar
    engine (per-partition bias); weighted reduce over h on the PE using a
    strided one-hot wv stationary so row qi of the scores PSUM tile is
    sum_h wv[h]*feats[h, :] (256-matmul accumulation per 128-qi tile).
  - softmax along free axis, attn transposed on PE, attn @ value on PE,
    1/rowsum folded into the output scale.

bv is omitted: it cancels in the softmax.
"""

import os
import sys

import numpy as np

for _p in ("/root/.axon_site", "/root/.axon_site/_ro/trn_rl_repo", "/opt/trn_rl_repo"):
    if os.path.isdir(_p) and _p not in sys.path:
        sys.path.append(_p)

import concourse.bacc as bacc
import concourse.bass as bass
import concourse.mybir as mybir
import concourse.tile as tile
from concourse.bass_utils import run_bass_kernel_spmd

B, LQ, LK = 4, 512, 512
QS, KS, H, DV = 512, 512, 256, 512
NCORES = 8
LQS = B * LQ // NCORES  # 256 query rows per core
QT = 128  # qi tile (partition dim)
F32 = mybir.dt.float32
# dtype of the tanh-feature tiles + one-hot wv stationary (the 1024 reduce
# matmuls): float16 runs 1 PE cycle/row vs float32's 4, with ~5e-4 element
# precision that washes out to ~1e-4 in the output.
DT_FEAT = mybir.dt.float16
NP_FEAT = np.float16
AF = mybir.ActivationFunctionType


def build():
    nc = bacc.Bacc("TRN2", target_bir_lowering=False, debug=False)

    queryT = nc.dram_tensor("queryT", [QS, LQS], F32, kind="ExternalInput")
    keyT = nc.dram_tensor("keyT", [KS, LK], F32, kind="ExternalInput")
    value = nc.dram_tensor("value", [LK, DV], F32, kind="ExternalInput")
    wqT = nc.dram_tensor("wqT", [QS, H], F32, kind="ExternalInput")
    wkT = nc.dram_tensor("wkT", [KS, H], F32, kind="ExternalInput")
    bqc = nc.dram_tensor("bqc", [128, 2], F32, kind="ExternalInput")
    bkc = nc.dram_tensor("bkc", [128, 2], F32, kind="ExternalInput")
    ohwv = nc.dram_tensor("ohwv", [128, 514], DT_FEAT, kind="ExternalInput")
    ident = nc.dram_tensor("ident", [128, 128], F32, kind="ExternalInput")
    out = nc.dram_tensor("out", [LQS, DV], F32, kind="ExternalOutput")

    with tile.TileContext(nc) as tc:
        with (
            tc.tile_pool(name="const", bufs=1) as constp,
            tc.tile_pool(name="feat", bufs=6) as featp,
            tc.tile_pool(name="sm", bufs=2) as smp,
            tc.tile_pool(name="ps_s", bufs=2, space="PSUM") as ps_s,
            tc.tile_pool(name="ps_t", bufs=2, space="PSUM") as ps_t,
            tc.tile_pool(name="ps_o", bufs=2, space="PSUM") as ps_o,
            tc.tile_pool(name="ps_p", bufs=2, space="PSUM") as ps_p,
        ):
            # ---- loads (d-major chunks of 128 partitions) ----
            qT_d = constp.tile([128, QS // 128, LQS], F32)
            nc.sync.dma_start(qT_d[:], queryT.ap().rearrange("(c p) q -> p c q", p=128))
            kT_d = constp.tile([128, KS // 128, LK], F32)
            nc.sync.dma_start(kT_d[:], keyT.ap().rearrange("(c p) k -> p c k", p=128))
            val = constp.tile([128, LK // 128, DV], F32)
            nc.sync.dma_start(val[:], value.ap().rearrange("(c p) d -> p c d", p=128))
            wq_s = constp.tile([128, QS // 128, H], F32)
            nc.sync.dma_start(wq_s[:], wqT.ap().rearrange("(c p) h -> p c h", p=128))
            wk_s = constp.tile([128, KS // 128, H], F32)
            nc.sync.dma_start(wk_s[:], wkT.ap().rearrange("(c p) h -> p c h", p=128))
            bq_s = constp.tile([128, 2], F32)
            nc.sync.dma_start(bq_s[:], bqc[:, :])
            bk_s = constp.tile([128, 2], F32)
            nc.sync.dma_start(bk_s[:], bkc[:, :])
            oh_s = constp.tile([128, 514], DT_FEAT)
            nc.sync.dma_start(oh_s[:], ohwv[:, :])
            id_s = constp.tile([128, 128], F32)
            nc.sync.dma_start(id_s[:], ident[:, :])

            # ---- projections: qTt [128, 2, LQS], kTt [128, 2, LK] ----
            qTt = constp.tile([128, 2, LQS], F32)
            kTt = constp.tile([128, 2, LK], F32)
            for hc in range(2):
                pq = ps_p.tile([128, LQS], F32, tag="proj")
                for dc in range(QS // 128):
                    nc.tensor.matmul(
                        pq[:],
                        wq_s[:, dc, hc * 128 : (hc + 1) * 128],
                        qT_d[:, dc, :],
                        start=(dc == 0),
                        stop=(dc == QS // 128 - 1),
                    )
                nc.scalar.activation(
                    qTt[:, hc, :], pq[:], AF.Identity, bias=bq_s[:, hc : hc + 1]
                )
                pk = ps_p.tile([128, LK], F32, tag="proj")
                for dc in range(KS // 128):
                    nc.tensor.matmul(
                        pk[:],
                        wk_s[:, dc, hc * 128 : (hc + 1) * 128],
                        kT_d[:, dc, :],
                        start=(dc == 0),
                        stop=(dc == KS // 128 - 1),
                    )
                nc.scalar.activation(
                    kTt[:, hc, :], pk[:], AF.Identity, bias=bk_s[:, hc : hc + 1]
                )

            # ---- main: scores, softmax, AV per 128-qi tile ----
            for t in range(LQS // QT):
                ps_scores = ps_s.tile([128, LK], F32, tag="scores")
                for qi in range(QT):
                    qg = t * QT + qi
                    for c in range(2):
                        ft = featp.tile([128, LK], DT_FEAT, tag="feats")
                        nc.scalar.activation(
                            ft[:], kTt[:, c, :], AF.Tanh, bias=qTt[:, c, qg : qg + 1]
                        )
                        nc.tensor.matmul(
                            ps_scores[:],
                            oh_s[:, c * 257 + 128 - qi : c * 257 + 256 - qi],
                            ft[:],
                            start=(qi == 0 and c == 0),
                            stop=(qi == QT - 1 and c == 1),
                        )
                negmax = smp.tile([128, 1], F32, tag="negmax")
                nc.vector.tensor_reduce(
                    negmax[:],
                    ps_scores[:],
                    axis=mybir.AxisListType.X,
                    op=mybir.AluOpType.max,
                    negate=True,
                )
                p = smp.tile([128, LK], F32, tag="p")
                nc.scalar.activation(p[:], ps_scores[:], AF.Exp, bias=negmax[:])
                ssum = smp.tile([128, 1], F32, tag="ssum")
                nc.vector.reduce_sum(ssum[:], p[:], axis=mybir.AxisListType.X)
                rinv = smp.tile([128, 1], F32, tag="rinv")
                nc.vector.reciprocal(rinv[:], ssum[:])

                ps_out = ps_o.tile([128, DV], F32, tag="av")
                for kc in range(LK // 128):
                    ptp = ps_t.tile([128, 128], F32, tag="ptp")
                    nc.tensor.transpose(ptp[:], p[:, kc * 128 : (kc + 1) * 128], id_s[:])
                    pts = featp.tile([128, 128], F32, tag="pts")
                    nc.vector.tensor_copy(pts[:], ptp[:])
                    nc.tensor.matmul(
                        ps_out[:],
                        pts[:],
                        val[:, kc, :],
                        start=(kc == 0),
                        stop=(kc == LK // 128 - 1),
                    )
                outs = smp.tile([128, DV], F32, tag="outs")
                nc.vector.tensor_scalar_mul(outs[:], ps_out[:], rinv[:])
                nc.sync.dma_start(out[t * QT : (t + 1) * QT, :], outs[:])

    nc.compile()
    return nc


_NC_CACHE = None


def _get_nc():
    global _NC_CACHE
    if _NC_CACHE is None:
        _NC_CACHE = build()
    return _NC_CACHE


def _make_in_maps(query, key, value, wq, bq, wk, bk, wv, bv):
    del bv  # cancels in softmax
    f = np.float32
    wqT = np.ascontiguousarray(np.asarray(wq, f).T)  # [QS, H]
    wkT = np.ascontiguousarray(np.asarray(wk, f).T)
    bq = np.asarray(bq, f)
    bk = np.asarray(bk, f)
    wv = np.asarray(wv, f)
    bqc = np.ascontiguousarray(bq.reshape(2, 128).T)  # [128, 2]
    bkc = np.ascontiguousarray(bk.reshape(2, 128).T)
    ohwv = np.zeros((128, 514), NP_FEAT)
    ohwv[:, 128] = wv[:128]
    ohwv[:, 385] = wv[128:]
    ident = np.eye(128, dtype=f)
    in_maps = []
    for core in range(NCORES):
        b, qh = divmod(core, NCORES // B)
        qsl = np.asarray(query[b, qh * LQS : (qh + 1) * LQS], f)  # [LQS, QS]
        in_maps.append(
            {
                "queryT": np.ascontiguousarray(qsl.T),
                "keyT": np.ascontiguousarray(np.asarray(key[b], f).T),
                "value": np.ascontiguousarray(np.asarray(value[b], f)),
                "wqT": wqT,
                "wkT": wkT,
                "bqc": bqc,
                "bkc": bkc,
                "ohwv": ohwv,
                "ident": ident,
            }
        )
    return in_maps


def _assemble(results):
    full = np.empty((B, LQ, DV), np.float32)
    for core in range(NCORES):
        b, qh = divmod(core, NCORES // B)
        full[b, qh * LQS : (qh + 1) * LQS, :] = results[core]["out"]
    return full


def run(inputs, trace=False, tmpdir=None):
    nc = _get_nc()
    in_maps = _make_in_maps(**inputs)
    kw = {}
    if trace:
        kw = dict(trace=True, tmpdir=tmpdir, trace_cores=list(range(NCORES)))
    res = run_bass_kernel_spmd(nc, in_maps, core_ids=list(range(NCORES)), **kw)
    return _assemble(res.results), res


def kernel(**inputs):
    out, _ = run(inputs, trace=False)
    return out


# revision 4
# speedup vs baseline: 1.7300x; 1.2925x over previous
"""Additive (Bahdanau) attention kernel for 8 TRN2 NeuronCores.

reference:
    q = query @ wq.T + bq            # [B, Lq, H]
    k = key  @ wk.T + bk             # [B, Lk, H]
    scores[b,qi,ki] = sum_h wv[h] * tanh(q[b,qi,h] + k[b,ki,h]) + bv
    out = softmax(scores, -1) @ value

Sharding: data-parallel over (B=4) x (Lq halves) -> 8 cores, each core
computes out[b, qh*256:(qh+1)*256, :] fully locally (no collectives).

Per-core device algorithm (shapes per core: Lq'=256, Lk=512, H=256):
  - projections via PE (host supplies pre-transposed queryT/keyT/wqT/wkT)
    -> qT [h, qi], kT [h, ki] with h on partitions (2 chunks of 128)
  - for each qi: feats_c = tanh(kT_c + qT_c[:, qi]) fused on the Scalar
    engine (per-partition bias); weighted reduce over h on the PE using a
    strided one-hot wv stationary so row qi of the scores PSUM tile is
    sum_h wv[h]*feats[h, :] (256-matmul accumulation per 128-qi tile).
  - softmax along free axis, attn transposed on PE, attn @ value on PE,
    1/rowsum folded into the output scale.

bv is omitted: it cancels in the softmax.
"""

import os
import sys

import numpy as np

for _p in ("/root/.axon_site", "/root/.axon_site/_ro/trn_rl_repo", "/opt/trn_rl_repo"):
    if os.path.isdir(_p) and _p not in sys.path:
        sys.path.append(_p)

import concourse.bacc as bacc
import concourse.bass as bass
import concourse.mybir as mybir
import concourse.tile as tile
from concourse.bass_utils import run_bass_kernel_spmd

B, LQ, LK = 4, 512, 512
QS, KS, H, DV = 512, 512, 256, 512
NCORES = 8
LQS = B * LQ // NCORES  # 256 query rows per core
QT = 128  # qi tile (partition dim)
F32 = mybir.dt.float32
# dtype of the tanh-feature tiles + one-hot wv stationary (the 1024 reduce
# matmuls): float16 runs 1 PE cycle/row vs float32's 4, with ~5e-4 element
# precision that washes out to ~1e-4 in the output.
DT_FEAT = mybir.dt.float16
NP_FEAT = np.float16
AF = mybir.ActivationFunctionType


def build():
    nc = bacc.Bacc("TRN2", target_bir_lowering=False, debug=False)

    queryT = nc.dram_tensor("queryT", [QS, LQS], F32, kind="ExternalInput")
    keyT = nc.dram_tensor("keyT", [KS, LK], F32, kind="ExternalInput")
    value = nc.dram_tensor("value", [LK, DV], F32, kind="ExternalInput")
    wqT = nc.dram_tensor("wqT", [QS, H], F32, kind="ExternalInput")
    wkT = nc.dram_tensor("wkT", [KS, H], F32, kind="ExternalInput")
    bqc = nc.dram_tensor("bqc", [128, 2], F32, kind="ExternalInput")
    bkc = nc.dram_tensor("bkc", [128, 2], F32, kind="ExternalInput")
    ohwv = nc.dram_tensor("ohwv", [128, 514], DT_FEAT, kind="ExternalInput")
    ident = nc.dram_tensor("ident", [128, 128], F32, kind="ExternalInput")
    out = nc.dram_tensor("out", [LQS, DV], F32, kind="ExternalOutput")

    with tile.TileContext(nc) as tc:
        with (
            tc.tile_pool(name="const", bufs=1) as constp,
            tc.tile_pool(name="feat", bufs=6) as featp,
            tc.tile_pool(name="sm", bufs=2) as smp,
            tc.tile_pool(name="ps_s", bufs=2, space="PSUM") as ps_s,
            tc.tile_pool(name="ps_t", bufs=2, space="PSUM") as ps_t,
            tc.tile_pool(name="ps_o", bufs=2, space="PSUM") as ps_o,
            tc.tile_pool(name="ps_p", bufs=2, space="PSUM") as ps_p,
        ):
            # ---- loads (d-major chunks of 128 partitions) ----
            qT_d = constp.tile([128, QS // 128, LQS], F32)
            nc.sync.dma_start(qT_d[:], queryT.ap().rearrange("(c p) q -> p c q", p=128))
            kT_d = constp.tile([128, KS // 128, LK], F32)
            nc.sync.dma_start(kT_d[:], keyT.ap().rearrange("(c p) k -> p c k", p=128))
            val = constp.tile([128, LK // 128, DV], F32)
            nc.sync.dma_start(val[:], value.ap().rearrange("(c p) d -> p c d", p=128))
            wq_s = constp.tile([128, QS // 128, H], F32)
            nc.sync.dma_start(wq_s[:], wqT.ap().rearrange("(c p) h -> p c h", p=128))
            wk_s = constp.tile([128, KS // 128, H], F32)
            nc.sync.dma_start(wk_s[:], wkT.ap().rearrange("(c p) h -> p c h", p=128))
            bq_s = constp.tile([128, 2], F32)
            nc.sync.dma_start(bq_s[:], bqc[:, :])
            bk_s = constp.tile([128, 2], F32)
            nc.sync.dma_start(bk_s[:], bkc[:, :])
            oh_s = constp.tile([128, 514], DT_FEAT)
            nc.sync.dma_start(oh_s[:], ohwv[:, :])
            id_s = constp.tile([128, 128], F32)
            nc.sync.dma_start(id_s[:], ident[:, :])

            # ---- projections: qTt [128, 2, LQS], kTt [128, 2, LK] ----
            qTt = constp.tile([128, 2, LQS], F32)
            kTt = constp.tile([128, 2, LK], F32)
            for hc in range(2):
                pq = ps_p.tile([128, LQS], F32, tag="proj")
                for dc in range(QS // 128):
                    nc.tensor.matmul(
                        pq[:],
                        wq_s[:, dc, hc * 128 : (hc + 1) * 128],
                        qT_d[:, dc, :],
                        start=(dc == 0),
                        stop=(dc == QS // 128 - 1),
                    )
                nc.scalar.activation(
                    qTt[:, hc, :], pq[:], AF.Identity, bias=bq_s[:, hc : hc + 1]
                )
                pk = ps_p.tile([128, LK], F32, tag="proj")
                for dc in range(KS // 128):
                    nc.tensor.matmul(
                        pk[:],
                        wk_s[:, dc, hc * 128 : (hc + 1) * 128],
                        kT_d[:, dc, :],
                        start=(dc == 0),
                        stop=(dc == KS // 128 - 1),
                    )
                nc.scalar.activation(
                    kTt[:, hc, :], pk[:], AF.Identity, bias=bk_s[:, hc : hc + 1]
                )

            # ---- main: scores, softmax, AV per 128-qi tile ----
            # Group GQ qi values: DVE precomputes kT + q_col into addbuf
            # (fp32 tensor_scalar = 2 elem/cycle/lane), ACT runs one big
            # Tanh over GQ*2*LK elements (amortizes the ~352-cycle ACT
            # instruction overhead), PE reduces per (qi, chunk).
            GQ = 4  # qi per group -> ACT free dim GQ*2*512 = 4096
            for t in range(LQS // QT):
                ps_scores = ps_s.tile([128, LK], F32, tag="scores")
                for g in range(QT // GQ):
                    addbuf = featp.tile([128, 2 * GQ, LK], F32, tag="addbuf", bufs=2)
                    for j in range(GQ):
                        qg = t * QT + g * GQ + j
                        for c in range(2):
                            nc.vector.tensor_scalar_add(
                                addbuf[:, j * 2 + c, :],
                                kTt[:, c, :],
                                qTt[:, c, qg : qg + 1],
                            )
                    ftg = featp.tile([128, 2 * GQ, LK], DT_FEAT, tag="feats", bufs=3)
                    nc.scalar.activation(ftg[:], addbuf[:], AF.Tanh)
                    for j in range(GQ):
                        qi = g * GQ + j
                        for c in range(2):
                            nc.tensor.matmul(
                                ps_scores[:],
                                oh_s[:, c * 257 + 128 - qi : c * 257 + 256 - qi],
                                ftg[:, j * 2 + c, :],
                                start=(qi == 0 and c == 0),
                                stop=(qi == QT - 1 and c == 1),
                            )
                negmax = smp.tile([128, 1], F32, tag="negmax")
                nc.vector.tensor_reduce(
                    negmax[:],
                    ps_scores[:],
                    axis=mybir.AxisListType.X,
                    op=mybir.AluOpType.max,
                    negate=True,
                )
                p = smp.tile([128, LK], F32, tag="p")
                nc.scalar.activation(p[:], ps_scores[:], AF.Exp, bias=negmax[:])
                ssum = smp.tile([128, 1], F32, tag="ssum")
                nc.vector.reduce_sum(ssum[:], p[:], axis=mybir.AxisListType.X)
                rinv = smp.tile([128, 1], F32, tag="rinv")
                nc.vector.reciprocal(rinv[:], ssum[:])

                ps_out = ps_o.tile([128, DV], F32, tag="av")
                for kc in range(LK // 128):
                    ptp = ps_t.tile([128, 128], F32, tag="ptp")
                    nc.tensor.transpose(ptp[:], p[:, kc * 128 : (kc + 1) * 128], id_s[:])
                    pts = featp.tile([128, 128], F32, tag="pts")
                    nc.vector.tensor_copy(pts[:], ptp[:])
                    nc.tensor.matmul(
                        ps_out[:],
                        pts[:],
                        val[:, kc, :],
                        start=(kc == 0),
                        stop=(kc == LK // 128 - 1),
                    )
                outs = smp.tile([128, DV], F32, tag="outs")
                nc.vector.tensor_scalar_mul(outs[:], ps_out[:], rinv[:])
                nc.sync.dma_start(out[t * QT : (t + 1) * QT, :], outs[:])

    nc.compile()
    return nc


_NC_CACHE = None


def _get_nc():
    global _NC_CACHE
    if _NC_CACHE is None:
        _NC_CACHE = build()
    return _NC_CACHE


def _make_in_maps(query, key, value, wq, bq, wk, bk, wv, bv):
    del bv  # cancels in softmax
    f = np.float32
    wqT = np.ascontiguousarray(np.asarray(wq, f).T)  # [QS, H]
    wkT = np.ascontiguousarray(np.asarray(wk, f).T)
    bq = np.asarray(bq, f)
    bk = np.asarray(bk, f)
    wv = np.asarray(wv, f)
    bqc = np.ascontiguousarray(bq.reshape(2, 128).T)  # [128, 2]
    bkc = np.ascontiguousarray(bk.reshape(2, 128).T)
    ohwv = np.zeros((128, 514), NP_FEAT)
    ohwv[:, 128] = wv[:128]
    ohwv[:, 385] = wv[128:]
    ident = np.eye(128, dtype=f)
    in_maps = []
    for core in range(NCORES):
        b, qh = divmod(core, NCORES // B)
        qsl = np.asarray(query[b, qh * LQS : (qh + 1) * LQS], f)  # [LQS, QS]
        in_maps.append(
            {
                "queryT": np.ascontiguousarray(qsl.T),
                "keyT": np.ascontiguousarray(np.asarray(key[b], f).T),
                "value": np.ascontiguousarray(np.asarray(value[b], f)),
                "wqT": wqT,
                "wkT": wkT,
                "bqc": bqc,
                "bkc": bkc,
                "ohwv": ohwv,
                "ident": ident,
            }
        )
    return in_maps


def _assemble(results):
    full = np.empty((B, LQ, DV), np.float32)
    for core in range(NCORES):
        b, qh = divmod(core, NCORES // B)
        full[b, qh * LQS : (qh + 1) * LQS, :] = results[core]["out"]
    return full


def run(inputs, trace=False, tmpdir=None):
    nc = _get_nc()
    in_maps = _make_in_maps(**inputs)
    kw = {}
    if trace:
        kw = dict(trace=True, tmpdir=tmpdir, trace_cores=list(range(NCORES)))
    res = run_bass_kernel_spmd(nc, in_maps, core_ids=list(range(NCORES)), **kw)
    return _assemble(res.results), res


def kernel(**inputs):
    out, _ = run(inputs, trace=False)
    return out


# revision 5
# speedup vs baseline: 1.7443x; 1.0083x over previous
"""Additive (Bahdanau) attention kernel for 8 TRN2 NeuronCores.

reference:
    q = query @ wq.T + bq            # [B, Lq, H]
    k = key  @ wk.T + bk             # [B, Lk, H]
    scores[b,qi,ki] = sum_h wv[h] * tanh(q[b,qi,h] + k[b,ki,h]) + bv
    out = softmax(scores, -1) @ value

Sharding: data-parallel over (B=4) x (Lq halves) -> 8 cores, each core
computes out[b, qh*256:(qh+1)*256, :] fully locally (no collectives).

Per-core device algorithm (shapes per core: Lq'=256, Lk=512, H=256):
  - projections via PE (host supplies pre-transposed queryT/keyT/wqT/wkT)
    -> qT [h, qi], kT [h, ki] with h on partitions (2 chunks of 128)
  - for each qi: feats_c = tanh(kT_c + qT_c[:, qi]) fused on the Scalar
    engine (per-partition bias); weighted reduce over h on the PE using a
    strided one-hot wv stationary so row qi of the scores PSUM tile is
    sum_h wv[h]*feats[h, :] (256-matmul accumulation per 128-qi tile).
  - softmax along free axis, attn transposed on PE, attn @ value on PE,
    1/rowsum folded into the output scale.

bv is omitted: it cancels in the softmax.
"""

import os
import sys

import numpy as np

for _p in ("/root/.axon_site", "/root/.axon_site/_ro/trn_rl_repo", "/opt/trn_rl_repo"):
    if os.path.isdir(_p) and _p not in sys.path:
        sys.path.append(_p)

import concourse.bacc as bacc
import concourse.bass as bass
import concourse.mybir as mybir
import concourse.tile as tile
from concourse.bass_utils import run_bass_kernel_spmd

B, LQ, LK = 4, 512, 512
QS, KS, H, DV = 512, 512, 256, 512
NCORES = 8
LQS = B * LQ // NCORES  # 256 query rows per core
QT = 128  # qi tile (partition dim)
F32 = mybir.dt.float32
# dtype of the tanh-feature tiles + one-hot wv stationary (the 1024 reduce
# matmuls): float16 runs 1 PE cycle/row vs float32's 4, with ~5e-4 element
# precision that washes out to ~1e-4 in the output.
DT_FEAT = mybir.dt.float16
NP_FEAT = np.float16
AF = mybir.ActivationFunctionType


def build():
    nc = bacc.Bacc("TRN2", target_bir_lowering=False, debug=False)

    queryT = nc.dram_tensor("queryT", [QS, LQS], F32, kind="ExternalInput")
    keyT = nc.dram_tensor("keyT", [KS, LK], F32, kind="ExternalInput")
    value = nc.dram_tensor("value", [LK, DV], F32, kind="ExternalInput")
    wqT = nc.dram_tensor("wqT", [QS, H], F32, kind="ExternalInput")
    wkT = nc.dram_tensor("wkT", [KS, H], F32, kind="ExternalInput")
    bqc = nc.dram_tensor("bqc", [128, 2], F32, kind="ExternalInput")
    bkc = nc.dram_tensor("bkc", [128, 2], F32, kind="ExternalInput")
    ohwv = nc.dram_tensor("ohwv", [128, 514], DT_FEAT, kind="ExternalInput")
    ident = nc.dram_tensor("ident", [128, 128], F32, kind="ExternalInput")
    out = nc.dram_tensor("out", [LQS, DV], F32, kind="ExternalOutput")

    with tile.TileContext(nc) as tc:
        with (
            tc.tile_pool(name="const", bufs=1) as constp,
            tc.tile_pool(name="feat", bufs=6) as featp,
            tc.tile_pool(name="sm", bufs=2) as smp,
            tc.tile_pool(name="ps_s", bufs=2, space="PSUM") as ps_s,
            tc.tile_pool(name="ps_t", bufs=2, space="PSUM") as ps_t,
            tc.tile_pool(name="ps_o", bufs=2, space="PSUM") as ps_o,
            tc.tile_pool(name="ps_p", bufs=2, space="PSUM") as ps_p,
        ):
            # ---- loads (d-major chunks of 128 partitions) ----
            qT_d = constp.tile([128, QS // 128, LQS], F32)
            nc.sync.dma_start(qT_d[:], queryT.ap().rearrange("(c p) q -> p c q", p=128))
            kT_d = constp.tile([128, KS // 128, LK], F32)
            nc.sync.dma_start(kT_d[:], keyT.ap().rearrange("(c p) k -> p c k", p=128))
            wq_s = constp.tile([128, QS // 128, H], F32)
            nc.sync.dma_start(wq_s[:], wqT.ap().rearrange("(c p) h -> p c h", p=128))
            wk_s = constp.tile([128, KS // 128, H], F32)
            nc.sync.dma_start(wk_s[:], wkT.ap().rearrange("(c p) h -> p c h", p=128))
            bq_s = constp.tile([128, 2], F32)
            nc.sync.dma_start(bq_s[:], bqc[:, :])
            bk_s = constp.tile([128, 2], F32)
            nc.sync.dma_start(bk_s[:], bkc[:, :])
            oh_s = constp.tile([128, 514], DT_FEAT)
            nc.sync.dma_start(oh_s[:], ohwv[:, :])
            id_s = constp.tile([128, 128], F32)
            nc.sync.dma_start(id_s[:], ident[:, :])

            # ---- projections: qTt [128, 2, LQS], kTt [128, 2, LK] ----
            qTt = constp.tile([128, 2, LQS], F32)
            kTt = constp.tile([128, 2, LK], DT_FEAT)
            for hc in range(2):
                pq = ps_p.tile([128, LQS], F32, tag="proj")
                for dc in range(QS // 128):
                    nc.tensor.matmul(
                        pq[:],
                        wq_s[:, dc, hc * 128 : (hc + 1) * 128],
                        qT_d[:, dc, :],
                        start=(dc == 0),
                        stop=(dc == QS // 128 - 1),
                    )
                nc.scalar.activation(
                    qTt[:, hc, :], pq[:], AF.Identity, bias=bq_s[:, hc : hc + 1]
                )
                pk = ps_p.tile([128, LK], F32, tag="proj")
                for dc in range(KS // 128):
                    nc.tensor.matmul(
                        pk[:],
                        wk_s[:, dc, hc * 128 : (hc + 1) * 128],
                        kT_d[:, dc, :],
                        start=(dc == 0),
                        stop=(dc == KS // 128 - 1),
                    )
                nc.scalar.activation(
                    kTt[:, hc, :], pk[:], AF.Identity, bias=bk_s[:, hc : hc + 1]
                )

            # value is only needed at AV time -- load it after the
            # projection inputs so it doesn't delay the critical path.
            val = constp.tile([128, LK // 128, DV], F32)
            nc.sync.dma_start(val[:], value.ap().rearrange("(c p) d -> p c d", p=128))

            # ---- main: scores, softmax, AV per 128-qi tile ----
            # Group GQ qi values: DVE precomputes kT + q_col into addbuf
            # (fp32 tensor_scalar = 2 elem/cycle/lane), ACT runs one big
            # Tanh over GQ*2*LK elements (amortizes the ~352-cycle ACT
            # instruction overhead), PE reduces per (qi, chunk).
            GQ = 4  # qi per group -> ACT free dim GQ*2*512 = 4096
            for t in range(LQS // QT):
                ps_scores = ps_s.tile([128, LK], F32, tag="scores")
                for g in range(QT // GQ):
                    addbuf = featp.tile([128, 2 * GQ, LK], DT_FEAT, tag="addbuf", bufs=3)
                    for j in range(GQ):
                        qg = t * QT + g * GQ + j
                        for c in range(2):
                            nc.vector.tensor_scalar_add(
                                addbuf[:, j * 2 + c, :],
                                kTt[:, c, :],
                                qTt[:, c, qg : qg + 1],
                            )
                    ftg = featp.tile([128, 2 * GQ, LK], DT_FEAT, tag="feats", bufs=3)
                    nc.scalar.activation(ftg[:], addbuf[:], AF.Tanh)
                    for j in range(GQ):
                        qi = g * GQ + j
                        for c in range(2):
                            nc.tensor.matmul(
                                ps_scores[:],
                                oh_s[:, c * 257 + 128 - qi : c * 257 + 256 - qi],
                                ftg[:, j * 2 + c, :],
                                start=(qi == 0 and c == 0),
                                stop=(qi == QT - 1 and c == 1),
                            )
                negmax = smp.tile([128, 1], F32, tag="negmax")
                nc.vector.tensor_reduce(
                    negmax[:],
                    ps_scores[:],
                    axis=mybir.AxisListType.X,
                    op=mybir.AluOpType.max,
                    negate=True,
                )
                p = smp.tile([128, LK], F32, tag="p")
                nc.scalar.activation(p[:], ps_scores[:], AF.Exp, bias=negmax[:])
                ssum = smp.tile([128, 1], F32, tag="ssum")
                nc.vector.reduce_sum(ssum[:], p[:], axis=mybir.AxisListType.X)
                rinv = smp.tile([128, 1], F32, tag="rinv")
                nc.vector.reciprocal(rinv[:], ssum[:])

                ps_out = ps_o.tile([128, DV], F32, tag="av")
                for kc in range(LK // 128):
                    ptp = ps_t.tile([128, 128], F32, tag="ptp")
                    nc.tensor.transpose(ptp[:], p[:, kc * 128 : (kc + 1) * 128], id_s[:])
                    pts = featp.tile([128, 128], F32, tag="pts")
                    nc.vector.tensor_copy(pts[:], ptp[:])
                    nc.tensor.matmul(
                        ps_out[:],
                        pts[:],
                        val[:, kc, :],
                        start=(kc == 0),
                        stop=(kc == LK // 128 - 1),
                    )
                outs = smp.tile([128, DV], F32, tag="outs")
                nc.vector.tensor_scalar_mul(outs[:], ps_out[:], rinv[:])
                nc.sync.dma_start(out[t * QT : (t + 1) * QT, :], outs[:])

    nc.compile()
    return nc


_NC_CACHE = None


def _get_nc():
    global _NC_CACHE
    if _NC_CACHE is None:
        _NC_CACHE = build()
    return _NC_CACHE


def _make_in_maps(query, key, value, wq, bq, wk, bk, wv, bv):
    del bv  # cancels in softmax
    f = np.float32
    wqT = np.ascontiguousarray(np.asarray(wq, f).T)  # [QS, H]
    wkT = np.ascontiguousarray(np.asarray(wk, f).T)
    bq = np.asarray(bq, f)
    bk = np.asarray(bk, f)
    wv = np.asarray(wv, f)
    bqc = np.ascontiguousarray(bq.reshape(2, 128).T)  # [128, 2]
    bkc = np.ascontiguousarray(bk.reshape(2, 128).T)
    ohwv = np.zeros((128, 514), NP_FEAT)
    ohwv[:, 128] = wv[:128]
    ohwv[:, 385] = wv[128:]
    ident = np.eye(128, dtype=f)
    in_maps = []
    for core in range(NCORES):
        b, qh = divmod(core, NCORES // B)
        qsl = np.asarray(query[b, qh * LQS : (qh + 1) * LQS], f)  # [LQS, QS]
        in_maps.append(
            {
                "queryT": np.ascontiguousarray(qsl.T),
                "keyT": np.ascontiguousarray(np.asarray(key[b], f).T),
                "value": np.ascontiguousarray(np.asarray(value[b], f)),
                "wqT": wqT,
                "wkT": wkT,
                "bqc": bqc,
                "bkc": bkc,
                "ohwv": ohwv,
                "ident": ident,
            }
        )
    return in_maps


def _assemble(results):
    full = np.empty((B, LQ, DV), np.float32)
    for core in range(NCORES):
        b, qh = divmod(core, NCORES // B)
        full[b, qh * LQS : (qh + 1) * LQS, :] = results[core]["out"]
    return full


def run(inputs, trace=False, tmpdir=None):
    nc = _get_nc()
    in_maps = _make_in_maps(**inputs)
    kw = {}
    if trace:
        kw = dict(trace=True, tmpdir=tmpdir, trace_cores=list(range(NCORES)))
    res = run_bass_kernel_spmd(nc, in_maps, core_ids=list(range(NCORES)), **kw)
    return _assemble(res.results), res


def kernel(**inputs):
    out, _ = run(inputs, trace=False)
    return out
